# revision 1
# baseline (speedup 1.0000x reference)
"""Trainium2 Bass kernel for nn_MixedPredictor (gnn_message_passing).

final[e] = softmax(gates)[0] * dot(h_user[src[e]], h_item[dst[e]])
         + softmax(gates)[1] * MLP(concat(h_user[src[e]], h_item[dst[e]]))

Strategy (8 NeuronCores, data-parallel over edges):
  - Edges are packed host-side into 8 * 31 macro tiles of 2048 slots. The
    gather bottleneck is SWDGE descriptor generation (~1.1 us per indirect
    DMA, max 128 rows each), so the packer exploits the contiguous-span
    semantics of indirect DMA: groups of 4 edges whose src (or dst) rows are
    consecutive (r..r+3) are fetched by ONE descriptor. Per macro: chunks
    0-3 = src-run slots (1 gather), 4-7 = dst-run slots (1 gather), 8-15 =
    classic slots (1 gather per chunk per side) -> 26 gathers instead of 32.
  - Each core gets the full node tables (random access) + its packed index
    arrays; outputs are unscrambled host-side via the slot->edge map.
  - dot-product on DVE (fused mult+reduce via scalar_tensor_tensor), fp32.
  - PE transposes chunks to feature-major; MLP + gate layer-1 run as fp32r
    matmuls with N=512 moving columns (full PE rate).
  - softmax over 2 gates == sigmoid(g1 - g0); tail (64-dim heads) is
    transposed back to edge-major and reduced on DVE so the final combine is
    lane-parallel and the store is in natural edge order.
"""

import numpy as np

import concourse.bass as bass
import concourse.bacc as bacc
import concourse.mybir as mybir
import concourse.tile as tile
from concourse.bass_utils import run_bass_kernel_spmd

N_CORES = 8
N_USERS = 100000
N_ITEMS = 100000
N_EDGES = 500000
D = 128

MACRO = 2048          # edges per macro tile
CHUNKS = MACRO // 128  # 16 chunks of 128 edges
NGROUPS = 4            # groups of 512 edges per macro (4 chunks each)
NMACRO = 31
E_CORE = NMACRO * MACRO  # 63488
E_PAD = N_CORES * E_CORE  # 507904

F32 = mybir.dt.float32
F32R = mybir.dt.float32r
I32 = mybir.dt.int32
AF = mybir.ActivationFunctionType
ALU = mybir.AluOpType

_CACHE = {}


def _find_runs(rows, avail_mask, need, n_rows=100000, L=4):
    """Sliding-greedy: emit L-consecutive-row runs while all rows occupied."""
    idx = np.nonzero(avail_mask)[0]
    order = np.argsort(rows[idx], kind="stable")
    sorted_eids = idx[order]
    counts = np.bincount(rows[sorted_eids], minlength=n_rows).astype(np.int64)
    offs = np.concatenate([[0], np.cumsum(counts)])
    taken = np.zeros(n_rows, dtype=np.int64)
    rem = counts.copy()
    runs_base, runs_edges = [], []
    r = 0
    while r <= n_rows - L and len(runs_base) < need:
        k = int(rem[r:r + L].min())
        if k <= 0:
            r += 1
            continue
        for _ in range(k):
            if len(runs_base) >= need:
                break
            runs_edges.append([sorted_eids[offs[r + j] + taken[r + j]] for j in range(L)])
            for j in range(L):
                taken[r + j] += 1
                rem[r + j] -= 1
            runs_base.append(r)
        if rem[r] <= 0:
            r += 1
    return np.array(runs_base, np.int32), np.array(runs_edges, np.int64)


def _pack(src, dst, nmac_tot, n_rows=100000):
    """4-row run-gather packing: per macro 256 src-runs (chunks 0-7),
    128 dst-runs (chunks 8-11), 512 classic slots (chunks 12-15)."""
    need_s = nmac_tot * 256
    need_d = nmac_tot * 128
    E = len(src)
    avail = np.ones(E, bool)
    sb, se = _find_runs(src, avail, need_s, n_rows)
    if len(sb) < need_s:
        raise RuntimeError(f"src run packing short: {len(sb)}/{need_s}")
    avail[se.ravel()] = False
    db, de = _find_runs(dst, avail, need_d, n_rows)
    if len(db) < need_d:
        raise RuntimeError(f"dst run packing short: {len(db)}/{need_d}")
    avail[de.ravel()] = False
    sb2, se2 = _find_runs(src, avail, nmac_tot * 128, n_rows, L=2)
    if len(sb2) < nmac_tot * 128:
        raise RuntimeError(f"src L2 packing short: {len(sb2)}/{nmac_tot * 128}")
    avail[se2.ravel()] = False
    rest = np.nonzero(avail)[0]
    if len(rest) > nmac_tot * 256:
        raise RuntimeError(f"classic slots overflow: {len(rest)}")
    return sb, se, db, de, sb2, se2, rest


def build_nc(nmacro=NMACRO):
    nc = bacc.Bacc(
        "TRN2",
        target_bir_lowering=False,
        debug=False,
        enable_asserts=False,
        num_devices=N_CORES,
    )

    hu = nc.dram_tensor("h_user", [N_USERS, D], F32, kind="ExternalInput").ap()
    hi = nc.dram_tensor("h_item", [N_ITEMS, D], F32, kind="ExternalInput").ap()
    srcs = nc.dram_tensor("srcc", [NMACRO, 128, 9], I32, kind="ExternalInput").ap()
    dsts = nc.dram_tensor("dstc", [NMACRO, 128, 13], I32, kind="ExternalInput").ap()
    w1d = nc.dram_tensor("W1", [256, 256], F32, kind="ExternalInput").ap()
    w2d = nc.dram_tensor("W2", [256, 128], F32, kind="ExternalInput").ap()
    w3d = nc.dram_tensor("W3", [128, 64], F32, kind="ExternalInput").ap()
    wg1d = nc.dram_tensor("Wg1", [256, 64], F32, kind="ExternalInput").ap()
    b1d = nc.dram_tensor("b1", [256], F32, kind="ExternalInput").ap()
    b2d = nc.dram_tensor("b2", [128], F32, kind="ExternalInput").ap()
    b3d = nc.dram_tensor("b3v", [64], F32, kind="ExternalInput").ap()
    bg1d = nc.dram_tensor("bg1v", [64], F32, kind="ExternalInput").ap()
    tailwd = nc.dram_tensor("tailw4", [512], F32, kind="ExternalInput").ap()
    identd = nc.dram_tensor("ident", [128, 128], F32, kind="ExternalInput").ap()
    b4d = nc.dram_tensor("b4s", [1], F32, kind="ExternalInput").ap()
    bg2dd = nc.dram_tensor("bg2d", [1], F32, kind="ExternalInput").ap()

    out = nc.dram_tensor("out", [E_CORE], F32, kind="ExternalOutput").ap()

    with tile.TileContext(nc) as tc:
        with (
            tc.tile_pool(name="const", bufs=1) as cp,
            tc.tile_pool(name="gather", bufs=2) as gp,
            tc.tile_pool(name="work", bufs=2) as wp,
            tc.tile_pool(name="psum", bufs=1, space="PSUM") as pp,
        ):
            # ---- constants ----
            w1k0 = cp.tile([128, 256], F32R, tag="w1k0")
            nc.sync.dma_start(out=w1k0[:], in_=w1d[0:128, :].bitcast(F32R))
            w1k1 = cp.tile([128, 256], F32R, tag="w1k1")
            nc.sync.dma_start(out=w1k1[:], in_=w1d[128:256, :].bitcast(F32R))
            w2k0 = cp.tile([128, 128], F32R, tag="w2k0")
            nc.sync.dma_start(out=w2k0[:], in_=w2d[0:128, :].bitcast(F32R))
            w2k1 = cp.tile([128, 128], F32R, tag="w2k1")
            nc.sync.dma_start(out=w2k1[:], in_=w2d[128:256, :].bitcast(F32R))
            w3t = cp.tile([128, 64], F32R, tag="w3t")
            nc.sync.dma_start(out=w3t[:], in_=w3d[:, :].bitcast(F32R))
            wg1k0 = cp.tile([128, 64], F32R, tag="wg1k0")
            nc.sync.dma_start(out=wg1k0[:], in_=wg1d[0:128, :].bitcast(F32R))
            wg1k1 = cp.tile([128, 64], F32R, tag="wg1k1")
            nc.sync.dma_start(out=wg1k1[:], in_=wg1d[128:256, :].bitcast(F32R))

            b1a = cp.tile([128, 1], F32, tag="b1a")
            nc.sync.dma_start(out=b1a[:], in_=b1d[0:128].rearrange("(p c) -> p c", c=1))
            b1b = cp.tile([128, 1], F32, tag="b1b")
            nc.sync.dma_start(out=b1b[:], in_=b1d[128:256].rearrange("(p c) -> p c", c=1))
            b2t = cp.tile([128, 1], F32, tag="b2t")
            nc.sync.dma_start(out=b2t[:], in_=b2d[:].rearrange("(p c) -> p c", c=1))
            b3sb = cp.tile([64, 1], F32, tag="b3sb")
            nc.sync.dma_start(out=b3sb[:], in_=b3d[:].rearrange("(p c) -> p c", c=1))
            bg1sb = cp.tile([64, 1], F32, tag="bg1sb")
            nc.sync.dma_start(out=bg1sb[:], in_=bg1d[:].rearrange("(p c) -> p c", c=1))
            b4t = cp.tile([128, 1], F32, tag="b4t")
            nc.sync.dma_start(out=b4t[:], in_=b4d.to_broadcast((128, 1)))
            bg2dt = cp.tile([128, 1], F32, tag="bg2dt")
            nc.sync.dma_start(out=bg2dt[:], in_=bg2dd.to_broadcast((128, 1)))
            tailw = cp.tile([128, 512], F32, tag="tailw")
            nc.sync.dma_start(
                out=tailw[:],
                in_=tailwd.rearrange("(p c) -> p c", p=1).to_broadcast((128, 512)),
            )
            ident = cp.tile([128, 128], F32, tag="ident")
            nc.sync.dma_start(out=ident[:], in_=identd[:, :])

            for m in range(nmacro):
                base = m * MACRO

                idx_s = gp.tile([128, 9], I32, tag="idx_s")
                nc.sync.dma_start(out=idx_s[:], in_=srcs[m, :, :])
                idx_d = gp.tile([128, 13], I32, tag="idx_d")
                nc.sync.dma_start(out=idx_d[:], in_=dsts[m, :, :])

                sg = gp.tile([128, MACRO], F32, tag="sg")
                dg = gp.tile([128, MACRO], F32, tag="dg")
                # src run-gathers: chunks 0-3 (base col 0), chunks 4-7 (base col 1)
                nc.gpsimd.indirect_dma_start(
                    out=sg[:, 0:512],
                    out_offset=None,
                    in_=hu,
                    in_offset=bass.IndirectOffsetOnAxis(ap=idx_s[:, 0:1], axis=0),
                )
                nc.gpsimd.indirect_dma_start(
                    out=sg[:, 512:1024],
                    out_offset=None,
                    in_=hu,
                    in_offset=bass.IndirectOffsetOnAxis(ap=idx_s[:, 1:2], axis=0),
                )
                # dst run-gather: chunks 8-11 (base col 0)
                nc.gpsimd.indirect_dma_start(
                    out=dg[:, 1024:1536],
                    out_offset=None,
                    in_=hi,
                    in_offset=bass.IndirectOffsetOnAxis(ap=idx_d[:, 0:1], axis=0),
                )
                # src L2 pair-run gather: chunks 12-13 (base col 2)
                nc.gpsimd.indirect_dma_start(
                    out=sg[:, 1536:1792],
                    out_offset=None,
                    in_=hu,
                    in_offset=bass.IndirectOffsetOnAxis(ap=idx_s[:, 2:3], axis=0),
                )
                # classic src gathers: chunks 8-11, 14-15 (idx cols 3-8)
                for k, c in enumerate((8, 9, 10, 11, 14, 15)):
                    nc.gpsimd.indirect_dma_start(
                        out=sg[:, c * 128 : (c + 1) * 128],
                        out_offset=None,
                        in_=hu,
                        in_offset=bass.IndirectOffsetOnAxis(ap=idx_s[:, 3 + k : 4 + k], axis=0),
                    )
                # classic dst gathers: chunks 0-7 and 12-15 (idx cols 1-12)
                for k in range(12):
                    c = k if k < 8 else 4 + k
                    nc.gpsimd.indirect_dma_start(
                        out=dg[:, c * 128 : (c + 1) * 128],
                        out_offset=None,
                        in_=hi,
                        in_offset=bass.IndirectOffsetOnAxis(ap=idx_d[:, 1 + k : 2 + k], axis=0),
                    )

                # ---- dot products, edge-major, fp32 ----
                dot_em = wp.tile([128, CHUNKS], F32, tag="dot_em")
                trash = wp.tile([128, 128], F32, tag="trash")
                for c in range(CHUNKS):
                    sl = slice(c * 128, (c + 1) * 128)
                    nc.vector.scalar_tensor_tensor(
                        out=trash[:],
                        in0=sg[:, sl],
                        scalar=1.0,
                        in1=dg[:, sl],
                        op0=ALU.mult,
                        op1=ALU.mult,
                        accum_out=dot_em[:, c : c + 1],
                    )

                final_em = wp.tile([128, CHUNKS], F32, tag="final_em")

                for g in range(NGROUPS):
                    # ---- transpose 4 chunks of S and D to feature-major ----
                    t_s = pp.tile([128, 512], F32, tag="t_s")
                    t_d = pp.tile([128, 512], F32, tag="t_d")
                    for j in range(4):
                        c = 4 * g + j
                        csl = slice(c * 128, (c + 1) * 128)
                        jsl = slice(j * 128, (j + 1) * 128)
                        nc.tensor.transpose(out=t_s[:, jsl], in_=sg[:, csl], identity=ident[:])
                        nc.tensor.transpose(out=t_d[:, jsl], in_=dg[:, csl], identity=ident[:])
                    xt_s = wp.tile([128, 512], F32R, tag="xt_s")
                    nc.scalar.copy(out=xt_s[:], in_=t_s[:])
                    xt_d = wp.tile([128, 512], F32R, tag="xt_d")
                    nc.vector.tensor_copy(out=xt_d[:], in_=t_d[:])

                    # ---- L1: H1 = relu(W1^T X + b1), 256 out feats ----
                    h1a = pp.tile([128, 512], F32, tag="h1a")
                    h1b = pp.tile([128, 512], F32, tag="h1b")
                    for mc, h1 in ((0, h1a), (1, h1b)):
                        msl = slice(mc * 128, (mc + 1) * 128)
                        nc.tensor.matmul(
                            out=h1[:], lhsT=(w1k0[:, msl]), rhs=(xt_s[:]),
                            start=True, stop=False,
                        )
                        nc.tensor.matmul(
                            out=h1[:], lhsT=(w1k1[:, msl]), rhs=(xt_d[:]),
                            start=False, stop=True,
                        )
                    h1sa = wp.tile([128, 512], F32R, tag="h1sa")
                    nc.scalar.activation(out=h1sa[:], in_=h1a[:], func=AF.Relu, bias=b1a[:])
                    h1sb = wp.tile([128, 512], F32R, tag="h1sb")
                    nc.scalar.activation(out=h1sb[:], in_=h1b[:], func=AF.Relu, bias=b1b[:])

                    # ---- L2: H2 = relu(W2^T H1 + b2), 128 out feats ----
                    h2p = pp.tile([128, 512], F32, tag="h2p")
                    nc.tensor.matmul(
                        out=h2p[:], lhsT=(w2k0[:]), rhs=(h1sa[:]),
                        start=True, stop=False,
                    )
                    nc.tensor.matmul(
                        out=h2p[:], lhsT=(w2k1[:]), rhs=(h1sb[:]),
                        start=False, stop=True,
                    )
                    h2s = wp.tile([128, 512], F32R, tag="h2s")
                    nc.scalar.activation(out=h2s[:], in_=h2p[:], func=AF.Relu, bias=b2t[:])

                    # ---- L3 (64 feats) + gate L1 (64 feats) ----
                    h3p = pp.tile([64, 512], F32, tag="h3p")
                    nc.tensor.matmul(
                        out=h3p[:], lhsT=(w3t[:]), rhs=(h2s[:]),
                        start=True, stop=True,
                    )
                    g1p = pp.tile([64, 512], F32, tag="g1p")
                    nc.tensor.matmul(
                        out=g1p[:], lhsT=(wg1k0[:]), rhs=(xt_s[:]),
                        start=True, stop=False,
                    )
                    nc.tensor.matmul(
                        out=g1p[:], lhsT=(wg1k1[:]), rhs=(xt_d[:]),
                        start=False, stop=True,
                    )
                    h3s = wp.tile([64, 512], F32, tag="h3s")
                    nc.scalar.activation(out=h3s[:], in_=h3p[:], func=AF.Relu, bias=b3sb[:])
                    g1rs = wp.tile([64, 512], F32, tag="g1rs")
                    nc.scalar.activation(out=g1rs[:], in_=g1p[:], func=AF.Relu, bias=bg1sb[:])

                    # ---- tail: back to edge-major (two 64-wide halves) ----
                    tt = pp.tile([128, 512], F32, tag="tt")
                    for j in range(4):
                        jsl = slice(j * 128, (j + 1) * 128)
                        nc.tensor.transpose(
                            out=tt[:, j * 128 : j * 128 + 64],
                            in_=h3s[:, jsl], identity=ident[0:64, 0:64],
                        )
                        nc.tensor.transpose(
                            out=tt[:, j * 128 + 64 : (j + 1) * 128],
                            in_=g1rs[:, jsl], identity=ident[0:64, 0:64],
                        )
                    prod = wp.tile([128, 512], F32, tag="prod")
                    nc.vector.tensor_tensor(out=prod[:], in0=tt[:], in1=tailw[:], op=ALU.mult)
                    red = wp.tile([128, 8], F32, tag="red")
                    nc.vector.reduce_sum(
                        out=red[:],
                        in_=prod[:].rearrange("p (c k) -> p c k", k=64),
                        axis=mybir.AxisListType.X,
                    )
                    red_v = red[:].rearrange("p (c two) -> p two c", two=2)
                    mlp_g = red_v[:, 0, :]
                    gd_g = red_v[:, 1, :]
                    dot_g = dot_em[:, 4 * g : 4 * g + 4]

                    sig = wp.tile([128, 4], F32, tag="sig")
                    nc.scalar.activation(out=sig[:], in_=gd_g, func=AF.Sigmoid, bias=bg2dt[:])
                    d1 = wp.tile([128, 4], F32, tag="d1")
                    nc.vector.tensor_sub(d1[:], mlp_g, dot_g)
                    sd = wp.tile([128, 4], F32, tag="sd")
                    nc.vector.scalar_tensor_tensor(
                        out=sd[:], in0=d1[:], scalar=b4t[:], in1=sig[:],
                        op0=ALU.add, op1=ALU.mult,
                    )
                    nc.vector.tensor_add(final_em[:, 4 * g : 4 * g + 4], sd[:], dot_g)

                nc.sync.dma_start(
                    out=out[base : base + MACRO].rearrange("(p c) -> p c", c=CHUNKS),
                    in_=final_em[:],
                )

    nc.compile()
    return nc


def _get_nc():
    if "nc" not in _CACHE:
        _CACHE["nc"] = build_nc()
    return _CACHE["nc"]


def kernel(h_user, h_item, src, dst,
           W1, b1, W2, b2, W3, b3, W4, b4,
           Wg1, bg1, Wg2, bg2, _trace=False):
    nc = _get_nc()

    h_user = np.ascontiguousarray(h_user, dtype=np.float32)
    h_item = np.ascontiguousarray(h_item, dtype=np.float32)
    src = np.asarray(src).astype(np.int64)
    dst = np.asarray(dst).astype(np.int64)

    nmac_tot = N_CORES * NMACRO
    sb, se, db, de, sb2, se2, rest = _pack(src, dst, nmac_tot)
    assert len(sb) == nmac_tot * 256 and len(db) == nmac_tot * 128
    assert len(sb2) == nmac_tot * 128
    # slot_edge [m, p, ch] = global edge id (-1 pad)
    slot_edge = np.full((nmac_tot, 128, 16), -1, dtype=np.int64)
    seL4 = se.reshape(nmac_tot, 2, 128, 4)
    slot_edge[:, :, 0:4] = seL4[:, 0]
    slot_edge[:, :, 4:8] = seL4[:, 1]
    slot_edge[:, :, 8:12] = de.reshape(nmac_tot, 128, 4)
    slot_edge[:, :, 12:14] = se2.reshape(nmac_tot, 128, 2)
    n_classic = nmac_tot * 128 * 2
    rest_pad = np.full(n_classic, -1, dtype=np.int64)
    rest_pad[: len(rest)] = rest
    slot_edge[:, :, 14:16] = rest_pad.reshape(nmac_tot, 128, 2)

    srcx = np.where(slot_edge >= 0, src[np.clip(slot_edge, 0, None)], 0)
    dstx = np.where(slot_edge >= 0, dst[np.clip(slot_edge, 0, None)], 0)
    srcc = np.zeros((nmac_tot, 128, 9), dtype=np.int32)
    dstc = np.zeros((nmac_tot, 128, 13), dtype=np.int32)
    sb4 = sb.reshape(nmac_tot, 2, 128)
    srcc[:, :, 0] = sb4[:, 0]
    srcc[:, :, 1] = sb4[:, 1]
    srcc[:, :, 2] = sb2.reshape(nmac_tot, 128)
    srcc[:, :, 3:7] = srcx[:, :, 8:12]
    srcc[:, :, 7:9] = srcx[:, :, 14:16]
    dstc[:, :, 0] = db.reshape(nmac_tot, 128)
    dstc[:, :, 1:9] = dstx[:, :, 0:8]
    dstc[:, :, 9:13] = dstx[:, :, 12:16]

    W1 = np.ascontiguousarray(W1, dtype=np.float32)
    W2 = np.ascontiguousarray(W2, dtype=np.float32)
    W3 = np.ascontiguousarray(W3, dtype=np.float32)
    Wg1 = np.ascontiguousarray(Wg1, dtype=np.float32)
    tailw4 = np.tile(
        np.concatenate([np.asarray(W4)[:, 0], np.asarray(Wg2)[:, 1] - np.asarray(Wg2)[:, 0]]),
        4,
    ).astype(np.float32)
    ident = np.eye(128, dtype=np.float32)
    b4s = np.asarray(b4, dtype=np.float32).reshape(1)
    bg2d = np.asarray([bg2[1] - bg2[0]], dtype=np.float32)

    common = {
        "h_user": h_user, "h_item": h_item,
        "W1": W1, "W2": W2, "W3": W3, "Wg1": Wg1,
        "b1": np.asarray(b1, dtype=np.float32),
        "b2": np.asarray(b2, dtype=np.float32),
        "b3v": np.asarray(b3, dtype=np.float32),
        "bg1v": np.asarray(bg1, dtype=np.float32),
        "tailw4": tailw4, "ident": ident,
        "b4s": b4s, "bg2d": bg2d,
    }
    in_maps = []
    for c in range(N_CORES):
        m = dict(common)
        m["srcc"] = srcc[c * NMACRO : (c + 1) * NMACRO]
        m["dstc"] = dstc[c * NMACRO : (c + 1) * NMACRO]
        in_maps.append(m)

    res = run_bass_kernel_spmd(
        nc, in_maps, core_ids=list(range(N_CORES)), trace=_trace
    )
    outs = np.concatenate([res.results[c]["out"] for c in range(N_CORES)])
    vals = outs.reshape(nmac_tot, 128, 16)
    final = np.zeros(N_EDGES, dtype=np.float32)
    mask = slot_edge >= 0
    final[slot_edge[mask]] = vals[mask]
    if _trace:
        kernel._last_result = res
    return final


kernel._last_result = None



# revision 4
# speedup vs baseline: 1.0385x; 1.0385x over previous
"""Trainium2 Bass kernel for nn_MixedPredictor (gnn_message_passing), v2.

final[e] = dot + sigmoid(gate)* (mlp - dot), with
  dot  = <h_user[src], h_item[dst]>
  mlp  = MLP_3(concat(s, d))
  gate = wg2diff . relu(Wg1a^T s + Wg1b^T d + bg1)   (softmax-of-2 == sigmoid)

Design (8 cores, data-parallel over edges):
  - Per-node combined tables (built host-side, bf16):
      U[i] = [h_user[i] (128) | Wg1a^T h_user[i] + bg1/2 (64) | pad (64)]
      V[j] = [h_item[j] (128) | Wg1b^T h_item[j] + bg1/2 (64) | pad (64)]
    512B rows -> one gather descriptor fetches embedding AND gate projection
    at the same modeled DMA cost as a bare 512B row.
  - Gathers via gpsimd.dma_gather (InstDMAGatherAnt, mlp library), 4096
    int16 indices per instruction, transpose=True -> rows land FEATURE-MAJOR
    ([128 feats, n_edges] bf16), eliminating all PE input transposes.
    int16 indices address within one of 4 25k-row table pieces; edges are
    sorted host-side into 16 (src_piece, dst_piece) classes and dealt
    round-robin across cores (pack.py logic inlined below).
  - Per 512-edge group: 9 bf16 matmuls (L1 x4, L2 x2, L3 x1, dot-reduce x1,
    heads x1), relu rides the PSUM->SBUF copies on ACT/DVE, gate layer-1 is
    just add+relu of the gathered projections.
  - Per-edge scalars (dot/mlp/gate) are transposed back to edge-major with
    nearly-free [3,128]->[128,3] PE transposes, tail combine runs per tile.
"""

import numpy as np
import ml_dtypes

import concourse.bass as bass
import concourse.bacc as bacc
import concourse.mybir as mybir
import concourse.tile as tile
from concourse import library_config
from concourse.bass_utils import run_bass_kernel_spmd

N_CORES = 8
N_USERS = 100000
N_ITEMS = 100000
N_EDGES = 500000
D = 128

NPIECE = 4
PIECE = 25000
NCLS = NPIECE * NPIECE
GW = 896                  # max columns per dma_gather (HW ucode cap; %128)
CW = 3968                 # class width (columns per class per core; %128)
EROW = 256                # bf16 elements per combined table row (512B)


def _plan(w):
    """Chunk w into gathers of <=GW (each %128==0) and 512/384-wide compute
    groups that never straddle a gather chunk. Returns (chunks, groups) where
    groups are (chunk_idx, offset_in_chunk, width)."""
    assert w % 128 == 0
    chunks, groups, left = [], [], w
    while left >= GW:
        chunks.append(GW)
        left -= GW
    if left:
        chunks.append(left)
    for t, cwidth in enumerate(chunks):
        off = 0
        while cwidth - off > 0:
            g = min(512, cwidth - off)
            if cwidth - off == 896:
                g = 512
            groups.append((t, off, g))
            off += g
    return chunks, groups

F32 = mybir.dt.float32
BF16 = mybir.dt.bfloat16
I16 = mybir.dt.int16
AF = mybir.ActivationFunctionType
ALU = mybir.AluOpType

_CACHE = {}


# ---------------------------------------------------------------- packing
def _pack(src, dst, w):
    cls = (src // PIECE) * NPIECE + (dst // PIECE)
    order = np.argsort(cls, kind="stable")
    cls_sorted = cls[order]
    bounds = np.searchsorted(cls_sorted, np.arange(NCLS + 1))
    slot_edge = np.full((NCLS, N_CORES, w), -1, np.int64)
    for c in range(NCLS):
        e = order[bounds[c]:bounds[c + 1]]
        for k in range(N_CORES):
            ek = e[k::N_CORES]
            assert len(ek) <= w, f"class {c} core {k}: {len(ek)} > {w}"
            slot_edge[c, k, :len(ek)] = ek
    sidx = np.where(slot_edge >= 0, src[np.clip(slot_edge, 0, None)] % PIECE, 0)
    didx = np.where(slot_edge >= 0, dst[np.clip(slot_edge, 0, None)] % PIECE, 0)
    return slot_edge, sidx.astype(np.int16), didx.astype(np.int16)


def _wrap(idx):
    """[..., W] int16 -> [..., 128, W//16]: i -> [i%16, i//16], x8 replicated."""
    w = idx.shape[-1]
    blk = idx.reshape(*idx.shape[:-1], w // 16, 16)
    blk = np.moveaxis(blk, -1, -2)
    return np.tile(blk, (*([1] * (idx.ndim - 1)), 8, 1)).astype(np.int16)


# ---------------------------------------------------------------- device
def build_nc(ncls=NCLS, w=CW):
    nc = bacc.Bacc(
        "TRN2",
        target_bir_lowering=False,
        debug=False,
        enable_asserts=False,
        num_devices=N_CORES,
    )

    ut = nc.dram_tensor("utab", [N_USERS, EROW], BF16, kind="ExternalInput").ap()
    vt = nc.dram_tensor("vtab", [N_ITEMS, EROW], BF16, kind="ExternalInput").ap()
    sidx = nc.dram_tensor("sidx", [ncls, 128, w // 16], I16, kind="ExternalInput").ap()
    didx = nc.dram_tensor("didx", [ncls, 128, w // 16], I16, kind="ExternalInput").ap()
    w1d = nc.dram_tensor("w1", [256, 256], BF16, kind="ExternalInput").ap()
    w2d = nc.dram_tensor("w2", [256, 128], BF16, kind="ExternalInput").ap()
    w3d = nc.dram_tensor("w3", [128, 64], BF16, kind="ExternalInput").ap()
    tailwd = nc.dram_tensor("tailw", [128, 3], BF16, kind="ExternalInput").ap()
    onesd = nc.dram_tensor("onesv", [128, 3], BF16, kind="ExternalInput").ap()
    identd = nc.dram_tensor("ident", [4, 4], F32, kind="ExternalInput").ap()
    b1d = nc.dram_tensor("b1v", [256], F32, kind="ExternalInput").ap()
    b2d = nc.dram_tensor("b2v", [128], F32, kind="ExternalInput").ap()
    b3d = nc.dram_tensor("b3v", [64], F32, kind="ExternalInput").ap()
    hbd = nc.dram_tensor("hbias", [4], F32, kind="ExternalInput").ap()

    out = nc.dram_tensor("out", [ncls * w], F32, kind="ExternalOutput").ap()

    with tile.TileContext(nc) as tc:
        with (
            tc.tile_pool(name="const", bufs=1) as cp,
            tc.tile_pool(name="gather", bufs=2) as gp,
            tc.tile_pool(name="work", bufs=2) as wp,
            tc.tile_pool(name="tail", bufs=2) as tp,
            tc.tile_pool(name="psum", bufs=1, space="PSUM") as pp,
            tc.tile_pool(name="psumT", bufs=2, space="PSUM") as ppt,
        ):
            nc.gpsimd.load_library(library_config.mlp)

            # ---- constants ----
            w1k = []
            for kc in range(2):
                for mc in range(2):
                    t = cp.tile([128, 128], BF16, tag=f"w1_{kc}{mc}")
                    nc.sync.dma_start(
                        out=t[:], in_=w1d[kc * 128:(kc + 1) * 128, mc * 128:(mc + 1) * 128]
                    )
                    w1k.append(t)
            w2k = []
            for kc in range(2):
                t = cp.tile([128, 128], BF16, tag=f"w2_{kc}")
                nc.sync.dma_start(out=t[:], in_=w2d[kc * 128:(kc + 1) * 128, :])
                w2k.append(t)
            w3t = cp.tile([128, 64], BF16, tag="w3t")
            nc.sync.dma_start(out=w3t[:], in_=w3d[:, :])
            tailw = cp.tile([128, 3], BF16, tag="tailw")
            nc.sync.dma_start(out=tailw[:], in_=tailwd[:, :])
            ones = cp.tile([128, 3], BF16, tag="ones")
            nc.sync.dma_start(out=ones[:], in_=onesd[:, :])
            ident = cp.tile([4, 4], F32, tag="ident")
            nc.sync.dma_start(out=ident[:], in_=identd[:, :])
            b1a = cp.tile([128, 1], F32, tag="b1a")
            nc.sync.dma_start(out=b1a[:], in_=b1d[0:128].rearrange("(p c) -> p c", c=1))
            b1b = cp.tile([128, 1], F32, tag="b1b")
            nc.sync.dma_start(out=b1b[:], in_=b1d[128:256].rearrange("(p c) -> p c", c=1))
            b2t = cp.tile([128, 1], F32, tag="b2t")
            nc.sync.dma_start(out=b2t[:], in_=b2d.rearrange("(p c) -> p c", c=1))
            b3t = cp.tile([64, 1], F32, tag="b3t")
            nc.sync.dma_start(out=b3t[:], in_=b3d.rearrange("(p c) -> p c", c=1))
            hb = cp.tile([4, 1], F32, tag="hb")
            nc.sync.dma_start(out=hb[:], in_=hbd.rearrange("(p c) -> p c", c=1))

            chunks, groups = _plan(w)
            cbase = np.concatenate([[0], np.cumsum(chunks)])
            for c in range(ncls):
                ps, pd = c // NPIECE, c % NPIECE

                six = gp.tile([128, w // 16], I16, tag="six")
                nc.sync.dma_start(out=six[:], in_=sidx[c])
                dix = gp.tile([128, w // 16], I16, tag="dix")
                nc.sync.dma_start(out=dix[:], in_=didx[c])

                us, ud = [], []
                for t, cwid in enumerate(chunks):
                    isl = slice(int(cbase[t]) // 16, int(cbase[t + 1]) // 16)
                    u = gp.tile([128, 2, cwid], BF16, tag=f"us{t}")
                    nc.gpsimd.dma_gather(
                        u[:], ut[ps * PIECE:(ps + 1) * PIECE, :], six[:, isl],
                        cwid, cwid, EROW, transpose=True,
                    )
                    us.append(u)
                    v = gp.tile([128, 2, cwid], BF16, tag=f"ud{t}")
                    nc.gpsimd.dma_gather(
                        v[:], vt[pd * PIECE:(pd + 1) * PIECE, :], dix[:, isl],
                        cwid, cwid, EROW, transpose=True,
                    )
                    ud.append(v)

                heads = tp.tile([4, w], F32, tag="heads")

                for gi, (ti, off, gw) in enumerate(groups):
                    gbase = int(cbase[ti]) + off  # column base within class
                    sl = slice(off, off + gw)
                    xs = us[ti][:, 0, sl]
                    xd = ud[ti][:, 0, sl]
                    gs = us[ti][0:64, 1, sl]
                    gd = ud[ti][0:64, 1, sl]

                    # dot-product input: elementwise product (bf16, DVE 4x)
                    prod = wp.tile([128, 512], BF16, tag="prod")[:, 0:gw]
                    nc.vector.tensor_tensor(out=prod, in0=xs, in1=xd, op=ALU.mult)

                    # gate layer 1: relu(gs + gd) (biases folded host-side)
                    h3g1 = wp.tile([128, 512], BF16, tag="h3g1")[:, 0:gw]
                    g1p = wp.tile([64, 512], BF16, tag="g1p")[:, 0:gw]
                    nc.vector.tensor_tensor(out=g1p, in0=gs, in1=gd, op=ALU.add)
                    nc.vector.tensor_scalar(
                        out=h3g1[64:128, :], in0=g1p, scalar1=0.0, scalar2=None,
                        op0=ALU.max,
                    )

                    # L1: h1 = relu(W1^T [xs; xd] + b1)
                    h1ap = pp.tile([128, 512], F32, tag="h1ap")[:, 0:gw]
                    h1bp = pp.tile([128, 512], F32, tag="h1bp")[:, 0:gw]
                    for mc, h1p in ((0, h1ap), (1, h1bp)):
                        nc.tensor.matmul(
                            out=h1p, lhsT=w1k[0 * 2 + mc][:], rhs=xs,
                            start=True, stop=False,
                        )
                        nc.tensor.matmul(
                            out=h1p, lhsT=w1k[1 * 2 + mc][:], rhs=xd,
                            start=False, stop=True,
                        )
                    h1sa = wp.tile([128, 512], BF16, tag="h1sa")[:, 0:gw]
                    nc.scalar.activation(out=h1sa, in_=h1ap, func=AF.Relu, bias=b1a[:])
                    h1sb = wp.tile([128, 512], BF16, tag="h1sb")[:, 0:gw]
                    nc.vector.tensor_scalar(
                        out=h1sb, in0=h1bp, scalar1=b1b[:], scalar2=0.0,
                        op0=ALU.add, op1=ALU.max,
                    )

                    # L2: h2 = relu(W2^T h1 + b2)
                    h2p = pp.tile([128, 512], F32, tag="h2p")[:, 0:gw]
                    nc.tensor.matmul(
                        out=h2p, lhsT=w2k[0][:], rhs=h1sa, start=True, stop=False,
                    )
                    nc.tensor.matmul(
                        out=h2p, lhsT=w2k[1][:], rhs=h1sb, start=False, stop=True,
                    )
                    h2s = wp.tile([128, 512], BF16, tag="h2s")[:, 0:gw]
                    nc.vector.tensor_scalar(
                        out=h2s, in0=h2p, scalar1=b2t[:], scalar2=0.0,
                        op0=ALU.add, op1=ALU.max,
                    )

                    # L3: h3 = relu(W3^T h2 + b3) -> h3g1[0:64]
                    h3p = pp.tile([128, 512], F32, tag="h3p")[:, 0:gw]
                    nc.tensor.matmul(
                        out=h3p[0:64, :], lhsT=w3t[:], rhs=h2s, start=True, stop=True,
                    )
                    nc.scalar.activation(
                        out=h3g1[0:64, :], in_=h3p[0:64, :], func=AF.Relu, bias=b3t[:]
                    )

                    # heads: [dot; mlp_pre; gate_pre] = 2 K-chained matmuls
                    hp = pp.tile([128, 512], F32, tag="hp")[:, 0:gw]
                    nc.tensor.matmul(
                        out=hp[0:3, :], lhsT=ones[:], rhs=prod, start=True, stop=False,
                    )
                    nc.tensor.matmul(
                        out=hp[0:3, :], lhsT=tailw[:], rhs=h3g1, start=False, stop=True,
                    )
                    # copy + per-row bias (0, b4, bg2diff) into class heads buffer
                    nc.scalar.activation(
                        out=heads[0:3, gbase:gbase + gw], in_=hp[0:3, :],
                        func=AF.Identity, bias=hb[0:3],
                    )

                # ---- tail: per class, back to edge-major ----
                tt = ppt.tile([128, 3 * (w // 128)], F32, tag="tt")
                for q in range(w // 128):
                    nc.tensor.matmul(
                        out=tt[:, 3 * q:3 * q + 3],
                        lhsT=heads[0:3, q * 128:(q + 1) * 128],
                        rhs=ident[0:3, 0:3],
                        is_transpose=True,
                    )
                ncols = w // 128
                # single full-tile copy PSUM->SBUF so downstream strided views
                # have one writer (tile dep analysis truncates many-writer
                # strided overlap checks)
                tts = tp.tile([128, 3 * ncols], F32, tag="tts")
                nc.scalar.activation(out=tts[:], in_=tt[:], func=AF.Copy)
                tt3 = tts[:].rearrange("p (q r) -> p q r", r=3)
                sig = tp.tile([128, ncols], F32, tag="sig")
                nc.scalar.activation(out=sig[:], in_=tt3[:, :, 2], func=AF.Sigmoid)
                d1 = tp.tile([128, ncols], F32, tag="d1")
                nc.vector.tensor_tensor(
                    out=d1[:], in0=tt3[:, :, 1], in1=tt3[:, :, 0], op=ALU.subtract
                )
                sd = tp.tile([128, ncols], F32, tag="sd")
                nc.vector.tensor_tensor(out=sd[:], in0=sig[:], in1=d1[:], op=ALU.mult)
                fin = tp.tile([128, ncols], F32, tag="fin")
                nc.vector.tensor_tensor(
                    out=fin[:], in0=sd[:], in1=tt3[:, :, 0], op=ALU.add
                )
                nc.sync.dma_start(
                    out=out[c * w:(c + 1) * w].rearrange("(p q) -> p q", q=ncols),
                    in_=fin[:],
                )

    nc.compile()
    return nc


def _get_nc(w=CW):
    if ("nc", w) not in _CACHE:
        _CACHE[("nc", w)] = build_nc(w=w)
    return _CACHE[("nc", w)]


def kernel(h_user, h_item, src, dst,
           W1, b1, W2, b2, W3, b3, W4, b4,
           Wg1, bg1, Wg2, bg2, _trace=False):
    bf = ml_dtypes.bfloat16
    h_user = np.asarray(h_user, np.float32)
    h_item = np.asarray(h_item, np.float32)
    src = np.asarray(src).astype(np.int64)
    dst = np.asarray(dst).astype(np.int64)
    W1 = np.asarray(W1, np.float32)
    Wg1 = np.asarray(Wg1, np.float32)
    bg1 = np.asarray(bg1, np.float32)

    # combined per-node tables (f32 math, bf16 storage)
    ut = np.zeros((N_USERS, EROW), bf)
    ut[:, 0:128] = h_user.astype(bf)
    ut[:, 128:192] = (h_user @ Wg1[:128] + bg1 / 2).astype(bf)
    vtab = np.zeros((N_ITEMS, EROW), bf)
    vtab[:, 0:128] = h_item.astype(bf)
    vtab[:, 128:192] = (h_item @ Wg1[128:] + bg1 / 2).astype(bf)

    # class width: default CW, bumped if any (class, core) bucket overflows
    clsv = (src // PIECE) * NPIECE + (dst // PIECE)
    maxcnt = int(np.bincount(clsv, minlength=NCLS).max())
    w = max(CW, ((maxcnt + N_CORES - 1) // N_CORES + 127) // 128 * 128)
    slot_edge, sidx, didx = _pack(src, dst, w)
    sidx_w = _wrap(sidx.reshape(NCLS * N_CORES, w)).reshape(NCLS, N_CORES, 128, w // 16)
    didx_w = _wrap(didx.reshape(NCLS * N_CORES, w)).reshape(NCLS, N_CORES, 128, w // 16)

    tailw = np.zeros((128, 3), bf)
    tailw[0:64, 1] = np.asarray(W4, np.float32)[:, 0].astype(bf)
    tailw[64:128, 2] = (np.asarray(Wg2)[:, 1] - np.asarray(Wg2)[:, 0]).astype(bf)
    ones3 = np.zeros((128, 3), bf)
    ones3[:, 0] = 1.0
    hbias = np.array(
        [0.0, np.asarray(b4, np.float32)[0], float(bg2[1] - bg2[0]), 0.0], np.float32
    )

    common = {
        "utab": ut, "vtab": vtab,
        "w1": W1.astype(bf), "w2": np.asarray(W2, np.float32).astype(bf),
        "w3": np.asarray(W3, np.float32).astype(bf),
        "tailw": tailw, "onesv": ones3,
        "ident": np.eye(4, dtype=np.float32),
        "b1v": np.asarray(b1, np.float32), "b2v": np.asarray(b2, np.float32),
        "b3v": np.asarray(b3, np.float32), "hbias": hbias,
    }
    in_maps = []
    for k in range(N_CORES):
        m = dict(common)
        m["sidx"] = sidx_w[:, k]
        m["didx"] = didx_w[:, k]
        in_maps.append(m)

    nc = _get_nc(w)
    res = run_bass_kernel_spmd(nc, in_maps, core_ids=list(range(N_CORES)), trace=_trace)

    final = np.zeros(N_EDGES, np.float32)
    ncols = w // 128
    j = np.arange(w)
    for k in range(N_CORES):
        o = np.asarray(res.results[k]["out"]).reshape(NCLS, 128, ncols)
        for c in range(NCLS):
            se = slot_edge[c, k]
            v = o[c, j % 128, j // 128]
            mask = se >= 0
            final[se[mask]] = v[mask]
    if _trace:
        kernel._last_result = res
    return final


kernel._last_result = None
